# revision 1
# baseline (speedup 1.0000x reference)
"""FANMoE HyperNet layer on 8 TRN2 NeuronCores.

Strategy: the reference materializes delta = h @ hW2 (512 x 394240, ~800MB).
Algebraically the einsum with x collapses per hypernet unit k:
    dyn[b, o] = sum_k h[b,k] * (x @ W2_k)[b, o]
where W2_k is the (IN, N*(DP+DN)) slice of hW2 for unit k. We shard the 8
cores as 4 expert-pairs x 2 batch-halves: each core handles 2 experts
(o-width 384) for 256 samples, streaming its hW2 slice (1/4 of columns)
once from HBM. Per-sample combination over k happens with per-partition
scalar_tensor_tensor ops on the vector engine. Matmuls run as a 3-term
fp16 hi/lo split (products are exact on the PE; fp16 denormals honored),
giving ~1e-5 relative accuracy at 1 cycle/row per term.

Host-side work is limited to layout prep (transposes, dtype splits,
slicing), sharding, and summing the per-core partial outputs.
"""
import math

import numpy as np

import concourse.bass as bass
import concourse.tile as tile
from concourse import mybir, bacc
from concourse.masks import make_identity

B, IN, OUT, COND, N, H = 512, 256, 256, 128, 8, 64
DP = 64
DN = 128
TPE = IN * DP + IN * DN + DN
BH = B // 2          # samples per core (batch half)
NBT = BH // 128      # 128-row tiles per core
W = 2 * (DP + DN)    # per-core output width: 2 experts x 192 = 384
dt = mybir.dt
F32 = dt.float32
F16 = dt.float16
AF = mybir.ActivationFunctionType
OP = mybir.AluOpType
INV2PI = 1.0 / (2.0 * math.pi)
N2PI = -2.0 * math.pi

_cache = {}


def _build(terms=3, repeat_main=1, ablate=()):
    nc = bacc.Bacc("TRN2", target_bir_lowering=False, debug=False)

    def din(name, shape, dty=F32):
        return nc.dram_tensor(name, shape, dty, kind="ExternalInput").ap()

    xT32 = din("xT32", (2, 128, BH))
    xh16 = din("xh16", (2, 128, BH), F16)
    if terms >= 3:
        xl16 = din("xl16", (2, 128, BH), F16)
    condT = din("condT", (COND, BH))
    # layout per k (terms>=3):
    #   [m0 | m1 | c0 | c1] with m_ic = [wph_ic | nh_ic], c_ic = [wpl_ic | nl_ic]
    # terms==2: [m0 | m1 | wpl0 | wpl1]
    WCH = 1536 if terms >= 3 else 1024
    w2hl = din("w2hl", (H, 128, WCH), F16)
    w2b = din("w2b", (2, H + 1, DN))
    wbase = din("wbase", (2, 128, W))
    hW1 = din("hW1", (COND, H))
    hb1 = din("hb1", (1, H))
    gW1 = din("gW1", (COND, 3 * N))
    gb1 = din("gb1", (3 * N, 1))
    gW2 = din("gW2", (3 * N, N))
    gb2 = din("gb2", (1, N))
    out = nc.dram_tensor("out", (BH, OUT), F32, kind="ExternalOutput").ap()

    with tile.TileContext(nc) as tc:
        with tc.tile_pool(name="const", bufs=1) as cp, \
             tc.tile_pool(name="tmp", bufs=4) as tp:
            ident = cp.tile([128, 128], F32)
            make_identity(nc, ident)
            ones = cp.tile([1, 128], F32)
            nc.vector.memset(ones, 1.0)
            halfpi = cp.tile([128, 1], F32)
            nc.vector.memset(halfpi, math.pi / 2)

            sxT = cp.tile([128, 2, BH], F32)
            sxh = cp.tile([128, 2, BH], F16)
            sxl = cp.tile([128, 2, BH], F16, name="sxl") if terms >= 3 else None
            for c in range(2):
                nc.sync.dma_start(sxT[:, c, :], xT32[c])
                nc.sync.dma_start(sxh[:, c, :], xh16[c])
                if terms >= 3:
                    nc.sync.dma_start(sxl[:, c, :], xl16[c])
            scT = cp.tile([COND, BH], F32)
            nc.sync.dma_start(scT, condT)
            swb = cp.tile([128, 2, W], F32)
            for c in range(2):
                nc.sync.dma_start(swb[:, c, :], wbase[c])
            sw2b = cp.tile([H + 1, 2, DN], F32)
            for e in range(2):
                nc.sync.dma_start(sw2b[:, e, :], w2b[e])
            shW1 = cp.tile([COND, H], F32)
            nc.sync.dma_start(shW1, hW1)
            shb1 = cp.tile([1, H], F32)
            nc.sync.dma_start(shb1, hb1)
            sgW1 = cp.tile([COND, 3 * N], F32)
            nc.sync.dma_start(sgW1, gW1)
            sgb1 = cp.tile([3 * N, 1], F32)
            nc.sync.dma_start(sgb1, gb1)
            sgW2 = cp.tile([3 * N, N], F32)
            nc.sync.dma_start(sgW2, gW2)
            sgb2 = cp.tile([1, N], F32)
            nc.sync.dma_start(sgb2, gb2)

            hTa = cp.tile([H + 1, NBT * 128], F32)
            nc.vector.memset(hTa[H:H + 1, :], 1.0)
            h_sb = [cp.tile([128, H], F32, name=f"h{t}") for t in range(NBT)]
            out_sb = [cp.tile([128, W], F32, name=f"os{t}") for t in range(NBT)]
            outf = [cp.tile([128, OUT], F32, name=f"of{t}") for t in range(NBT)]
            gw_sb = [cp.tile([128, 2], F32, name=f"gw{t}") for t in range(NBT)]

            # ---------------- prologue: gating, hypernet h, base ----------
            with tc.tile_pool(name="pps", bufs=2, space="PSUM") as pps:
                g1 = pps.tile([3 * N, BH], F32, tag="g1", bufs=1)
                nc.tensor.matmul(g1, sgW1, scT, start=True, stop=True)
                g1s = cp.tile([3 * N, BH], F32)
                nc.scalar.activation(g1s, g1, AF.Relu, bias=sgb1)

                for bt in range(NBT):
                    bs = slice(bt * 128, bt * 128 + 128)
                    hp = pps.tile([128, H], F32, tag="hp", bufs=1)
                    nc.tensor.matmul(hp, scT[:, bs], shW1,
                                     start=True, stop=False)
                    nc.tensor.matmul(hp, ones, shb1, start=False, stop=True)
                    nc.scalar.activation(h_sb[bt], hp, AF.Relu)
                    ht = pps.tile([H, 128], F32, tag="ht", bufs=1)
                    nc.tensor.transpose(ht, h_sb[bt], ident)
                    nc.scalar.copy(hTa[0:H, bs], ht)

                    lg = pps.tile([128, N], F32, tag="lg", bufs=1)
                    nc.tensor.matmul(lg, g1s[:, bs], sgW2,
                                     start=True, stop=False)
                    nc.tensor.matmul(lg, ones, sgb2, start=False, stop=True)
                    nmx = tp.tile([128, 1], F32, tag="nmx")
                    nc.vector.tensor_reduce(nmx, lg, axis=mybir.AxisListType.X,
                                            op=OP.max, negate=True)
                    ex = tp.tile([128, N], F32, tag="ex")
                    nc.scalar.activation(ex, lg, AF.Exp, bias=nmx)
                    sm = tp.tile([128, 1], F32, tag="sm")
                    nc.vector.tensor_reduce(sm, ex, axis=mybir.AxisListType.X,
                                            op=OP.add)
                    rv = tp.tile([128, 1], F32, tag="rv")
                    nc.vector.reciprocal(rv, sm)
                    nc.vector.tensor_scalar_mul(gw_sb[bt], ex[:, 0:2], rv)

                    bp = pps.tile([128, W], F32, tag="bp", bufs=2)
                    nc.tensor.matmul(bp, sxT[:, 0, bs], swb[:, 0, :],
                                     start=True, stop=False)
                    nc.tensor.matmul(bp, sxT[:, 1, bs], swb[:, 1, :],
                                     start=False, stop=False)
                    nc.tensor.matmul(bp[:, 128:256], hTa[:, bs],
                                     sw2b[:, 0, :], start=False, stop=False)
                    nc.tensor.matmul(bp[:, 256:384], hTa[:, bs],
                                     sw2b[:, 1, :], start=False, stop=True)
                    nc.scalar.copy(out_sb[bt], bp)

            # ---------------- main loop over hypernet units k --------------
            with tc.tile_pool(name="wp", bufs=4) as wp, \
                 tc.tile_pool(name="mps", bufs=8, space="PSUM") as mps:

              def _main_body():
                # k-pairs; within a pair, matmuls are stationary-major so
                # LDWEIGHTS amortizes over up to 4 consecutive matmuls.
                for k0 in range(0, H, 2):
                    wts, pss = [], []
                    for k in (k0, k0 + 1):
                        wt = wp.tile([128, WCH], F16, tag="w", name=f"w{k}")
                        if "dma" not in ablate:
                            nc.sync.dma_start(wt, w2hl[k])
                        else:
                            nc.gpsimd.memset(wt, 0.0)
                        wts.append(wt)
                    mm = nc.tensor.matmul
                    for bt in range(NBT):
                        bs = slice(bt * 128, bt * 128 + 128)
                        prs = [mps.tile([128, W], F32, tag="ps", name=f"ps{j}")
                               for j in range(2)]
                        pss.append(prs)
                        if "mm" in ablate:
                            for ps in prs:
                                nc.scalar.memzero(ps)
                            continue
                        # stationary-major: xh0, xh1, xl0, xl1
                        for j, wt in enumerate(wts):
                            mm(prs[j], sxh[:, 0, bs], wt[:, 0:384],
                               start=True, stop=False)
                            mm(prs[j], sxh[:, 0, bs], wt[:, 768:1152],
                               start=False, stop=False)
                        for j, wt in enumerate(wts):
                            mm(prs[j], sxh[:, 1, bs], wt[:, 384:768],
                               start=False, stop=False)
                            mm(prs[j], sxh[:, 1, bs], wt[:, 1152:1536],
                               start=False, stop=False)
                        for j, wt in enumerate(wts):
                            mm(prs[j], sxl[:, 0, bs], wt[:, 0:384],
                               start=False, stop=False)
                        for j, wt in enumerate(wts):
                            mm(prs[j], sxl[:, 1, bs], wt[:, 384:768],
                               start=False, stop=True)
                    if "stt" not in ablate:
                        for bt in range(NBT):
                            for j, k in enumerate((k0, k0 + 1)):
                                nc.vector.scalar_tensor_tensor(
                                    out_sb[bt], pss[bt][j],
                                    h_sb[bt][:, k:k + 1],
                                    out_sb[bt], op0=OP.mult, op1=OP.add)


              if repeat_main == 1:
                  _main_body()
              else:
                  with tc.For_i(0, repeat_main, 1):
                      _main_body()

            # ---------------- epilogue: sin/cos/relu, gate, store ----------
            def sin_reduced(v, outname):
                """sin(v) via range reduction robust to trunc- or
                round-to-nearest float->int conversion."""
                t1 = tp.tile([128, DP], F32, tag="t1")
                nc.vector.tensor_scalar_mul(t1, v, INV2PI)
                ti = tp.tile([128, DP], dt.int32, tag="ti")
                nc.vector.tensor_copy(ti, t1)
                tf = tp.tile([128, DP], F32, tag="tf")
                nc.vector.tensor_copy(tf, ti)
                r = tp.tile([128, DP], F32, tag="r")
                nc.vector.scalar_tensor_tensor(r, tf, N2PI, v,
                                               op0=OP.mult, op1=OP.add)
                m = tp.tile([128, DP], F32, tag="m")
                nc.vector.tensor_scalar(m, r, math.pi, None, op0=OP.is_gt)
                nc.vector.scalar_tensor_tensor(r, m, N2PI, r,
                                               op0=OP.mult, op1=OP.add)
                nc.vector.tensor_scalar(m, r, -math.pi, None, op0=OP.is_lt)
                nc.vector.scalar_tensor_tensor(r, m, -N2PI, r,
                                               op0=OP.mult, op1=OP.add)
                sv = tp.tile([128, DP], F32, tag=outname, name=outname)
                nc.scalar.activation(sv, r, AF.Sin)
                return sv

            for bt in range(NBT):
                for e in range(2):
                    th = out_sb[bt][:, e * DP:(e + 1) * DP]
                    g = gw_sb[bt][:, e:e + 1]

                    sv = sin_reduced(th, "sv")
                    u = tp.tile([128, DP], F32, tag="u")
                    nc.vector.tensor_scalar_add(u, th, math.pi / 2)
                    cv = sin_reduced(u, "cv")

                    nn = tp.tile([128, DN], F32, tag="nn")
                    nc.scalar.activation(
                        nn, out_sb[bt][:, 128 + e * DN:128 + (e + 1) * DN],
                        AF.Relu)

                    if e == 0:
                        nc.vector.tensor_scalar_mul(outf[bt][:, 0:DP], cv, g)
                        nc.vector.tensor_scalar_mul(outf[bt][:, DP:2 * DP],
                                                    sv, g)
                        nc.vector.tensor_scalar_mul(outf[bt][:, 2 * DP:OUT],
                                                    nn, g)
                    else:
                        stt = nc.vector.scalar_tensor_tensor
                        stt(outf[bt][:, 0:DP], cv, g, outf[bt][:, 0:DP],
                            op0=OP.mult, op1=OP.add)
                        stt(outf[bt][:, DP:2 * DP], sv, g,
                            outf[bt][:, DP:2 * DP], op0=OP.mult, op1=OP.add)
                        stt(outf[bt][:, 2 * DP:OUT], nn, g,
                            outf[bt][:, 2 * DP:OUT], op0=OP.mult, op1=OP.add)
                nc.sync.dma_start(out[bt * 128:bt * 128 + 128, :], outf[bt])

    nc.finalize()
    return nc


def _host_prep(x, cond, base_wp, base_wn, base_bn, hW1, hb1, hW2, hb2,
               gW1, gb1, gW2, gb2, terms=3):
    """Build the 8 per-core input maps (layout prep + sharding only)."""
    f32 = np.float32
    W2r = np.asarray(hW2, f32).reshape(H, N, TPE)
    wpW = W2r[:, :, :IN * DP].reshape(H, N, IN, DP)
    wnW = W2r[:, :, IN * DP:IN * DP + IN * DN].reshape(H, N, IN, DN)
    bnW = W2r[:, :, IN * DP + IN * DN:]                    # (H, N, DN)
    hb2r = np.asarray(hb2, f32).reshape(N, TPE)
    hwp = hb2r[:, :IN * DP].reshape(N, IN, DP)
    hwn = hb2r[:, IN * DP:IN * DP + IN * DN].reshape(N, IN, DN)
    hbn = hb2r[:, IN * DP + IN * DN:]                      # (N, DN)

    base_wp = np.asarray(base_wp, f32)
    base_wn = np.asarray(base_wn, f32)
    base_bn = np.asarray(base_bn, f32)
    x = np.asarray(x, f32)
    cond = np.asarray(cond, f32)
    gW2 = np.asarray(gW2, f32)
    gb2 = np.asarray(gb2, f32)

    common = dict(
        hW1=np.ascontiguousarray(hW1, f32),
        hb1=np.asarray(hb1, f32).reshape(1, H).copy(),
        gW1=np.ascontiguousarray(gW1, f32),
        gb1=np.asarray(gb1, f32).reshape(3 * N, 1).copy(),
    )

    # per batch-half arrays
    halves = []
    for hb in range(2):
        bs = slice(hb * BH, (hb + 1) * BH)
        xT = np.ascontiguousarray(x[bs].T)                 # (IN, BH)
        xh = xT.astype(np.float16)
        d = dict(
            xT32=np.ascontiguousarray(xT.reshape(2, 128, BH)),
            xh16=np.ascontiguousarray(xh.reshape(2, 128, BH)),
            condT=np.ascontiguousarray(cond[bs].T),
        )
        if terms >= 3:
            xl = (xT - xh.astype(f32)).astype(np.float16)
            d["xl16"] = np.ascontiguousarray(xl.reshape(2, 128, BH))
        halves.append(d)

    pairs = []
    for p in range(4):
        e0, e1 = 2 * p, 2 * p + 1
        wpcat = np.concatenate([wpW[:, e0], wpW[:, e1]], axis=-1)  # (H,IN,128)
        ncat = np.concatenate([wnW[:, e0], wnW[:, e1]], axis=-1)   # (H,IN,256)
        # per k rows p: [wph ic0|nh ic0|wph ic1|nh ic1|wpl ic0|wpl ic1]
        wpc = wpcat.reshape(H, 2, 128, 128)                # [k, ic, p, 128]
        nc_ = ncat.reshape(H, 2, 128, 256)
        wp_hi = wpc.astype(np.float16)
        wp_lo = (wpc - wp_hi.astype(f32)).astype(np.float16)
        n_hi = nc_.astype(np.float16)
        n_lo = (nc_ - n_hi.astype(f32)).astype(np.float16)
        parts = [wp_hi[:, 0], n_hi[:, 0], wp_hi[:, 1], n_hi[:, 1],
                 wp_lo[:, 0], n_lo[:, 0], wp_lo[:, 1], n_lo[:, 1]]
        w2hl = np.concatenate(parts, axis=-1)
        w2b = np.stack([
            np.concatenate([bnW[:, e], (base_bn[e] + hbn[e])[None, :]], axis=0)
            for e in (e0, e1)])                            # (2, 65, DN)
        wb = np.concatenate(
            [base_wp[e0] + hwp[e0], base_wp[e1] + hwp[e1],
             base_wn[e0] + hwn[e0], base_wn[e1] + hwn[e1]],
            axis=-1)                                       # (IN, 384)
        perm = [e0, e1] + [j for j in range(N) if j not in (e0, e1)]
        pairs.append(dict(
            w2hl=np.ascontiguousarray(w2hl),
            w2b=np.ascontiguousarray(w2b),
            wbase=np.ascontiguousarray(wb.reshape(2, 128, W)),
            gW2=np.ascontiguousarray(gW2[:, perm]),
            gb2=np.ascontiguousarray(gb2[perm].reshape(1, N)),
        ))

    in_maps = []
    for c in range(8):
        p, hb = c // 2, c % 2
        m = dict(common)
        m.update(halves[hb])
        m.update(pairs[p])
        in_maps.append(m)
    return in_maps


def _make_runner(nc, n_cores=8):
    """Compile once; reusable executor for per-core input maps."""
    import jax
    from jax.sharding import Mesh, PartitionSpec
    from jax.experimental.shard_map import shard_map
    from concourse.bass2jax import (_bass_exec_p, install_neuronx_cc_hook,
                                    partition_id_tensor)

    install_neuronx_cc_hook()
    pname = nc.partition_id_tensor.name if nc.partition_id_tensor else None
    in_names, out_names, out_avals, zero_outs = [], [], [], []
    for alloc in nc.m.functions[0].allocations:
        if not isinstance(alloc, mybir.MemoryLocationSet):
            continue
        name = alloc.memorylocations[0].name
        if alloc.kind == "ExternalInput":
            if name != pname:
                in_names.append(name)
        elif alloc.kind == "ExternalOutput":
            out_names.append(name)
            shape = tuple(alloc.tensor_shape)
            dtype = mybir.dt.np(alloc.dtype)
            out_avals.append(jax.core.ShapedArray(shape, dtype))
            zero_outs.append(np.zeros(shape, dtype))
    n_params = len(in_names)
    n_outs = len(out_avals)
    all_names = in_names + out_names + ([pname] if pname else [])

    def _body(*args):
        operands = list(args)
        if pname is not None:
            operands.append(partition_id_tensor())
        outs = _bass_exec_p.bind(
            *operands, out_avals=tuple(out_avals), in_names=tuple(all_names),
            out_names=tuple(out_names), lowering_input_output_aliases=(),
            sim_require_finite=True, sim_require_nnan=True, nc=nc)
        return tuple(outs)

    devices = jax.devices()[:n_cores]
    mesh = Mesh(np.asarray(devices), ("core",))
    in_specs = (PartitionSpec("core"),) * (n_params + n_outs)
    out_specs = (PartitionSpec("core"),) * n_outs
    donate = tuple(range(n_params, n_params + n_outs))
    sharded = jax.jit(
        shard_map(_body, mesh=mesh, in_specs=in_specs, out_specs=out_specs,
                  check_rep=False),
        donate_argnums=donate, keep_unused=True)

    staged = {}

    def _concat(in_maps):
        return [
            np.concatenate([np.asarray(in_maps[c][in_names[i]])
                            for c in range(n_cores)], axis=0)
            for i in range(n_params)
        ]

    def run(in_maps):
        if in_maps is None:
            concat_in = staged["dev"]
        else:
            concat_in = _concat(in_maps)
        zeros = [np.zeros((n_cores * z.shape[0], *z.shape[1:]), z.dtype)
                 for z in zero_outs]
        outs = sharded(*concat_in, *zeros)
        arr = np.asarray(outs[0]).reshape(n_cores, *out_avals[0].shape)
        return [{out_names[0]: arr[c]} for c in range(n_cores)]

    def preload(in_maps):
        import jax
        staged["dev"] = [jax.device_put(a) for a in _concat(in_maps)]
        for a in staged["dev"]:
            a.block_until_ready()

    run.preload = preload
    return run


def kernel(**inputs):
    terms = _cache.get("terms", 3)
    if "run" not in _cache:
        nc = _build(terms)
        _cache["nc"] = nc
        _cache["run"] = _make_runner(nc)
    in_maps = _host_prep(**inputs, terms=terms)
    results = _cache["run"](in_maps)
    out = np.zeros((B, OUT), np.float32)
    for c in range(8):
        hb = c % 2
        out[hb * BH:(hb + 1) * BH] += results[c]["out"]
    return out



# revision 2
# speedup vs baseline: 3.2730x; 3.2730x over previous
"""FANMoE HyperNet layer on 8 TRN2 NeuronCores.

Strategy: the reference materializes delta = h @ hW2 (512 x 394240, ~800MB).
Algebraically the einsum with x collapses per hypernet unit k:
    dyn[b, o] = sum_k h[b,k] * (x @ W2_k)[b, o]
We shard the 8 cores as 4 expert-pairs x 2 batch-halves: each core handles
2 experts (o-width 384) for 256 samples, streaming its hW2 slice (1/4 of
columns, fp16) once from HBM.

v1 change vs baseline: single fp16 term (tolerance gate is 2e-2; measured
rel_fro ~4e-4) and the per-k h-weighting is folded into the PE stationary:
    z_k[i, b] = fp16(xh[i, b] * h[b, k])
built by 2x-mode fp16 tensor_tensor ops on DVE against a partition-broadcast
copy of h. PSUM then accumulates all 64 k-terms of x@W directly (start at
k=0, stop at k=63), eliminating the 1x-mode scalar_tensor_tensor combine
that dominated DVE time. Weights stream HBM->SBUF in 4-k chunks inside the
loop, double-buffered.

Host-side work is limited to layout prep (transposes, dtype splits,
slicing), sharding, and summing the per-core partial outputs.
"""
import math

import numpy as np

import concourse.bass as bass
import concourse.tile as tile
from concourse import mybir, bacc
from concourse.masks import make_identity

B, IN, OUT, COND, N, H = 512, 256, 256, 128, 8, 64
DP = 64
DN = 128
TPE = IN * DP + IN * DN + DN
BH = B // 2          # samples per core (batch half)
NBT = BH // 128      # 128-row tiles per core
W = 2 * (DP + DN)    # per-core output width: 2 experts x 192 = 384
KC = 4               # hypernet units per weight DMA chunk
dt = mybir.dt
F32 = dt.float32
F16 = dt.float16
AF = mybir.ActivationFunctionType
OP = mybir.AluOpType
INV2PI = 1.0 / (2.0 * math.pi)
N2PI = -2.0 * math.pi

_cache = {"terms": 1}


def _build(terms=1, repeat_main=1, ablate=(), preload=False, bcast="pb"):
    nc = bacc.Bacc("TRN2", target_bir_lowering=False, debug=False)

    def din(name, shape, dty=F32):
        return nc.dram_tensor(name, shape, dty, kind="ExternalInput").ap()

    xT32 = din("xT32", (2, 128, BH))
    xh16 = din("xh16", (2, 128, BH), F16)
    condT = din("condT", (COND, BH))
    # weights per k: [128, 768]; cols [ic*384 : ic*384+384] belong to input
    # chunk ic, inner order [wp_e0 64 | wp_e1 64 | wn_e0 128 | wn_e1 128].
    # Stored pre-chunked as (H//KC, 128, KC, 768) so each DMA is contiguous
    # per partition.
    w2h = din("w2h", (H // KC, 128, KC, 768), F16)
    w2b = din("w2b", (2, H + 1, DN))
    wbase = din("wbase", (2, 128, W))
    hW1 = din("hW1", (COND, H))
    hb1 = din("hb1", (1, H))
    gW1 = din("gW1", (COND, 3 * N))
    gb1 = din("gb1", (3 * N, 1))
    gW2 = din("gW2", (3 * N, N))
    gb2 = din("gb2", (1, N))
    out = nc.dram_tensor("out", (BH, OUT), F32, kind="ExternalOutput").ap()

    with tile.TileContext(nc) as tc:
        with tc.tile_pool(name="const", bufs=1) as cp, \
             tc.tile_pool(name="tmp", bufs=4) as tp, \
             tc.tile_pool(name="acc", bufs=1, space="PSUM") as accp:
            ident = cp.tile([128, 128], F32)
            make_identity(nc, ident)
            ones = cp.tile([1, 128], F32)
            nc.vector.memset(ones, 1.0)

            sxT = cp.tile([128, 2, BH], F32)
            sxh = cp.tile([128, 2, BH], F16)
            for c in range(2):
                nc.sync.dma_start(sxT[:, c, :], xT32[c])
                nc.sync.dma_start(sxh[:, c, :], xh16[c])
            scT = cp.tile([COND, BH], F32)
            nc.sync.dma_start(scT, condT)
            swb = cp.tile([128, 2, W], F32)
            for c in range(2):
                nc.sync.dma_start(swb[:, c, :], wbase[c])
            sw2b = cp.tile([H + 1, 2, DN], F32)
            for e in range(2):
                nc.sync.dma_start(sw2b[:, e, :], w2b[e])
            shW1 = cp.tile([COND, H], F32)
            nc.sync.dma_start(shW1, hW1)
            shb1 = cp.tile([1, H], F32)
            nc.sync.dma_start(shb1, hb1)
            sgW1 = cp.tile([COND, 3 * N], F32)
            nc.sync.dma_start(sgW1, gW1)
            sgb1 = cp.tile([3 * N, 1], F32)
            nc.sync.dma_start(sgb1, gb1)
            sgW2 = cp.tile([3 * N, N], F32)
            nc.sync.dma_start(sgW2, gW2)
            sgb2 = cp.tile([1, N], F32)
            nc.sync.dma_start(sgb2, gb2)

            if preload:
                wall = cp.tile([128, H // KC, KC, 768], F16)
                for j in range(H // KC):
                    nc.sync.dma_start(wall[:, j], w2h[j])

            hTa = cp.tile([H + 1, NBT * 128], F32)
            nc.vector.memset(hTa[H:H + 1, :], 1.0)
            hT16 = cp.tile([H, NBT * 128], F16)
            hflat = cp.tile([1, H * BH], F16)
            hbc = cp.tile([128, H, BH], F16)
            h_sb = [cp.tile([128, H], F32, name=f"h{t}") for t in range(NBT)]
            bsb = cp.tile([128, 2, W], F32)
            outf = [cp.tile([128, OUT], F32, name=f"of{t}") for t in range(NBT)]
            gw_sb = [cp.tile([128, 2], F32, name=f"gw{t}") for t in range(NBT)]

            # psum accumulators for the main loop, one bank per batch tile
            ps = [accp.tile([128, W], F32, name=f"acc{t}", tag=f"acc{t}",
                            bufs=1) for t in range(NBT)]

            # ---------------- prologue: gating, hypernet h, base ----------
            with tc.tile_pool(name="pps", bufs=2, space="PSUM") as pps:
                g1 = pps.tile([3 * N, BH], F32, tag="g1", bufs=1)
                nc.tensor.matmul(g1, sgW1, scT, start=True, stop=True)
                g1s = cp.tile([3 * N, BH], F32)
                nc.scalar.activation(g1s, g1, AF.Relu, bias=sgb1)

                for bt in range(NBT):
                    bs = slice(bt * 128, bt * 128 + 128)
                    hp = pps.tile([128, H], F32, tag="hp", bufs=1)
                    nc.tensor.matmul(hp, scT[:, bs], shW1,
                                     start=True, stop=False)
                    nc.tensor.matmul(hp, ones, shb1, start=False, stop=True)
                    nc.scalar.activation(h_sb[bt], hp, AF.Relu)
                    ht = pps.tile([H, 128], F32, tag="ht", bufs=1)
                    nc.tensor.transpose(ht, h_sb[bt], ident)
                    nc.scalar.copy(hTa[0:H, bs], ht)

                    lg = pps.tile([128, N], F32, tag="lg", bufs=1)
                    nc.tensor.matmul(lg, g1s[:, bs], sgW2,
                                     start=True, stop=False)
                    nc.tensor.matmul(lg, ones, sgb2, start=False, stop=True)
                    nmx = tp.tile([128, 1], F32, tag="nmx")
                    nc.vector.tensor_reduce(nmx, lg, axis=mybir.AxisListType.X,
                                            op=OP.max, negate=True)
                    ex = tp.tile([128, N], F32, tag="ex")
                    nc.scalar.activation(ex, lg, AF.Exp, bias=nmx)
                    sm = tp.tile([128, 1], F32, tag="sm")
                    nc.vector.tensor_reduce(sm, ex, axis=mybir.AxisListType.X,
                                            op=OP.add)
                    rv = tp.tile([128, 1], F32, tag="rv")
                    nc.vector.reciprocal(rv, sm)
                    nc.vector.tensor_scalar_mul(gw_sb[bt], ex[:, 0:2], rv)

                    bp = pps.tile([128, W], F32, tag="bp", bufs=2)
                    nc.tensor.matmul(bp, sxT[:, 0, bs], swb[:, 0, :],
                                     start=True, stop=False)
                    nc.tensor.matmul(bp, sxT[:, 1, bs], swb[:, 1, :],
                                     start=False, stop=False)
                    nc.tensor.matmul(bp[:, 128:256], hTa[:, bs],
                                     sw2b[:, 0, :], start=False, stop=False)
                    nc.tensor.matmul(bp[:, 256:384], hTa[:, bs],
                                     sw2b[:, 1, :], start=False, stop=True)
                    nc.scalar.copy(bsb[:, bt, :], bp)

                # broadcast h (fp16, transposed) to all 128 partitions:
                # hbc[p, k, b] = h[b, k] for every p
                nc.vector.tensor_copy(hT16, hTa[0:H, :])
                nc.sync.dma_start(hflat[0:1].rearrange("o (k b) -> o k b",
                                                       k=H, b=BH), hT16)
                if bcast == "pb":
                    nc.gpsimd.partition_broadcast(hbc, hflat[0:1])
                else:
                    nc.sync.dma_start(hbc[0:1], hflat)
                    p = 1
                    while p < 128:
                        n = min(p, 128 - p)
                        nc.sync.dma_start(
                            hbc[p:p + n],
                            hbc[0:n] if n != p else hbc[0:p])
                        p += n

            # ---------------- main loop over hypernet units k --------------
            with tc.tile_pool(name="wp", bufs=3) as wp, \
                 tc.tile_pool(name="zp", bufs=4) as zp:

              zfix = None
              if "z" in ablate:
                  zfix = [cp.tile([128, 2, BH], F16, name=f"zf{j}")
                          for j in range(2)]
                  for z in zfix:
                      nc.vector.tensor_copy(z, sxh)
              if "mm" in ablate:
                  for bt in range(NBT):
                      nc.vector.memset(ps[bt], 0.0)

              def _main_body():
                for kc in range(0, H, KC):
                    if not preload:
                        wt = wp.tile([128, KC, 768], F16, tag="w")
                        if "dma" not in ablate:
                            nc.sync.dma_start(wt, w2h[kc // KC])
                    else:
                        wt = wall[:, kc // KC]
                    for j in range(KC):
                        k = kc + j
                        if "z" not in ablate:
                            zt = zp.tile([128, 2, BH], F16, tag="z")
                            for ic in range(2):
                                nc.vector.tensor_tensor(
                                    zt[:, ic, :], sxh[:, ic, :], hbc[:, k, :],
                                    op=OP.mult)
                        else:
                            zt = zfix[k % 2]
                        if "mm" in ablate:
                            continue
                        for bt in range(NBT):
                            bs = slice(bt * 128, bt * 128 + 128)
                            for ic in range(2):
                                nc.tensor.matmul(
                                    ps[bt], zt[:, ic, bs],
                                    wt[:, j, ic * 384:ic * 384 + 384],
                                    start=(k == 0 and ic == 0),
                                    stop=(k == H - 1 and ic == 1))

              if repeat_main == 1:
                  _main_body()
              else:
                  with tc.For_i(0, repeat_main, 1):
                      _main_body()

            # ---------------- epilogue: sin/cos/relu, gate, store ----------
            def sin_reduced(v, outname):
                """sin(v) via range reduction robust to trunc- or
                round-to-nearest float->int conversion."""
                t1 = tp.tile([128, DP], F32, tag="t1")
                nc.vector.tensor_scalar_mul(t1, v, INV2PI)
                ti = tp.tile([128, DP], dt.int32, tag="ti")
                nc.vector.tensor_copy(ti, t1)
                tf = tp.tile([128, DP], F32, tag="tf")
                nc.vector.tensor_copy(tf, ti)
                r = tp.tile([128, DP], F32, tag="r")
                nc.vector.scalar_tensor_tensor(r, tf, N2PI, v,
                                               op0=OP.mult, op1=OP.add)
                m = tp.tile([128, DP], F32, tag="m")
                nc.vector.tensor_scalar(m, r, math.pi, None, op0=OP.is_gt)
                nc.vector.scalar_tensor_tensor(r, m, N2PI, r,
                                               op0=OP.mult, op1=OP.add)
                nc.vector.tensor_scalar(m, r, -math.pi, None, op0=OP.is_lt)
                nc.vector.scalar_tensor_tensor(r, m, -N2PI, r,
                                               op0=OP.mult, op1=OP.add)
                sv = tp.tile([128, DP], F32, tag=outname, name=outname)
                nc.scalar.activation(sv, r, AF.Sin)
                return sv

            for bt in range(NBT):
                comb = tp.tile([128, W], F32, tag="comb")
                nc.vector.tensor_tensor(comb, ps[bt], bsb[:, bt, :],
                                        op=OP.add)
                for e in range(2):
                    th = comb[:, e * DP:(e + 1) * DP]
                    g = gw_sb[bt][:, e:e + 1]

                    sv = sin_reduced(th, "sv")
                    u = tp.tile([128, DP], F32, tag="u")
                    nc.vector.tensor_scalar_add(u, th, math.pi / 2)
                    cv = sin_reduced(u, "cv")

                    nn = tp.tile([128, DN], F32, tag="nn")
                    nc.scalar.activation(
                        nn, comb[:, 128 + e * DN:128 + (e + 1) * DN],
                        AF.Relu)

                    if e == 0:
                        nc.vector.tensor_scalar_mul(outf[bt][:, 0:DP], cv, g)
                        nc.vector.tensor_scalar_mul(outf[bt][:, DP:2 * DP],
                                                    sv, g)
                        nc.vector.tensor_scalar_mul(outf[bt][:, 2 * DP:OUT],
                                                    nn, g)
                    else:
                        stt = nc.vector.scalar_tensor_tensor
                        stt(outf[bt][:, 0:DP], cv, g, outf[bt][:, 0:DP],
                            op0=OP.mult, op1=OP.add)
                        stt(outf[bt][:, DP:2 * DP], sv, g,
                            outf[bt][:, DP:2 * DP], op0=OP.mult, op1=OP.add)
                        stt(outf[bt][:, 2 * DP:OUT], nn, g,
                            outf[bt][:, 2 * DP:OUT], op0=OP.mult, op1=OP.add)
                nc.sync.dma_start(out[bt * 128:bt * 128 + 128, :], outf[bt])

    nc.finalize()
    return nc


def _host_prep(x, cond, base_wp, base_wn, base_bn, hW1, hb1, hW2, hb2,
               gW1, gb1, gW2, gb2, terms=1):
    """Build the 8 per-core input maps (layout prep + sharding only)."""
    f32 = np.float32
    W2r = np.asarray(hW2, f32).reshape(H, N, TPE)
    wpW = W2r[:, :, :IN * DP].reshape(H, N, IN, DP)
    wnW = W2r[:, :, IN * DP:IN * DP + IN * DN].reshape(H, N, IN, DN)
    bnW = W2r[:, :, IN * DP + IN * DN:]                    # (H, N, DN)
    hb2r = np.asarray(hb2, f32).reshape(N, TPE)
    hwp = hb2r[:, :IN * DP].reshape(N, IN, DP)
    hwn = hb2r[:, IN * DP:IN * DP + IN * DN].reshape(N, IN, DN)
    hbn = hb2r[:, IN * DP + IN * DN:]                      # (N, DN)

    base_wp = np.asarray(base_wp, f32)
    base_wn = np.asarray(base_wn, f32)
    base_bn = np.asarray(base_bn, f32)
    x = np.asarray(x, f32)
    cond = np.asarray(cond, f32)
    gW2 = np.asarray(gW2, f32)
    gb2 = np.asarray(gb2, f32)

    common = dict(
        hW1=np.ascontiguousarray(hW1, f32),
        hb1=np.asarray(hb1, f32).reshape(1, H).copy(),
        gW1=np.ascontiguousarray(gW1, f32),
        gb1=np.asarray(gb1, f32).reshape(3 * N, 1).copy(),
    )

    # per batch-half arrays
    halves = []
    for hb in range(2):
        bs = slice(hb * BH, (hb + 1) * BH)
        xT = np.ascontiguousarray(x[bs].T)                 # (IN, BH)
        xh = xT.astype(np.float16)
        halves.append(dict(
            xT32=np.ascontiguousarray(xT.reshape(2, 128, BH)),
            xh16=np.ascontiguousarray(xh.reshape(2, 128, BH)),
            condT=np.ascontiguousarray(cond[bs].T),
        ))

    pairs = []
    for p in range(4):
        e0, e1 = 2 * p, 2 * p + 1
        wpcat = np.concatenate([wpW[:, e0], wpW[:, e1]], axis=-1)  # (H,IN,128)
        ncat = np.concatenate([wnW[:, e0], wnW[:, e1]], axis=-1)   # (H,IN,256)
        full = np.concatenate([wpcat, ncat], axis=-1)      # (H, IN, 384)
        # (H, 2, 128, 384) -> per k: [128, 768] with ic-major columns,
        # then chunk k by KC with k inner so each DMA chunk is contiguous
        # per partition: (H//KC, 128, KC, 768)
        w2h = full.reshape(H, 2, 128, 384).transpose(0, 2, 1, 3) \
                  .reshape(H // KC, KC, 128, 768).transpose(0, 2, 1, 3)
        w2b = np.stack([
            np.concatenate([bnW[:, e], (base_bn[e] + hbn[e])[None, :]], axis=0)
            for e in (e0, e1)])                            # (2, 65, DN)
        wb = np.concatenate(
            [base_wp[e0] + hwp[e0], base_wp[e1] + hwp[e1],
             base_wn[e0] + hwn[e0], base_wn[e1] + hwn[e1]],
            axis=-1)                                       # (IN, 384)
        perm = [e0, e1] + [j for j in range(N) if j not in (e0, e1)]
        pairs.append(dict(
            w2h=np.ascontiguousarray(w2h.astype(np.float16)),
            w2b=np.ascontiguousarray(w2b),
            wbase=np.ascontiguousarray(wb.reshape(2, 128, W)),
            gW2=np.ascontiguousarray(gW2[:, perm]),
            gb2=np.ascontiguousarray(gb2[perm].reshape(1, N)),
        ))

    in_maps = []
    for c in range(8):
        p, hb = c // 2, c % 2
        m = dict(common)
        m.update(halves[hb])
        m.update(pairs[p])
        in_maps.append(m)
    return in_maps


def _make_runner(nc, n_cores=8):
    """Compile once; reusable executor for per-core input maps."""
    import jax
    from jax.sharding import Mesh, PartitionSpec
    from jax.experimental.shard_map import shard_map
    from concourse.bass2jax import (_bass_exec_p, install_neuronx_cc_hook,
                                    partition_id_tensor)

    install_neuronx_cc_hook()
    pname = nc.partition_id_tensor.name if nc.partition_id_tensor else None
    in_names, out_names, out_avals, zero_outs = [], [], [], []
    for alloc in nc.m.functions[0].allocations:
        if not isinstance(alloc, mybir.MemoryLocationSet):
            continue
        name = alloc.memorylocations[0].name
        if alloc.kind == "ExternalInput":
            if name != pname:
                in_names.append(name)
        elif alloc.kind == "ExternalOutput":
            out_names.append(name)
            shape = tuple(alloc.tensor_shape)
            dtype = mybir.dt.np(alloc.dtype)
            out_avals.append(jax.core.ShapedArray(shape, dtype))
            zero_outs.append(np.zeros(shape, dtype))
    n_params = len(in_names)
    n_outs = len(out_avals)
    all_names = in_names + out_names + ([pname] if pname else [])

    def _body(*args):
        operands = list(args)
        if pname is not None:
            operands.append(partition_id_tensor())
        outs = _bass_exec_p.bind(
            *operands, out_avals=tuple(out_avals), in_names=tuple(all_names),
            out_names=tuple(out_names), lowering_input_output_aliases=(),
            sim_require_finite=True, sim_require_nnan=True, nc=nc)
        return tuple(outs)

    devices = jax.devices()[:n_cores]
    mesh = Mesh(np.asarray(devices), ("core",))
    in_specs = (PartitionSpec("core"),) * (n_params + n_outs)
    out_specs = (PartitionSpec("core"),) * n_outs
    donate = tuple(range(n_params, n_params + n_outs))
    sharded = jax.jit(
        shard_map(_body, mesh=mesh, in_specs=in_specs, out_specs=out_specs,
                  check_rep=False),
        donate_argnums=donate, keep_unused=True)

    staged = {}

    def _concat(in_maps):
        return [
            np.concatenate([np.asarray(in_maps[c][in_names[i]])
                            for c in range(n_cores)], axis=0)
            for i in range(n_params)
        ]

    def run(in_maps):
        if in_maps is None:
            concat_in = staged["dev"]
        else:
            concat_in = _concat(in_maps)
        zeros = [np.zeros((n_cores * z.shape[0], *z.shape[1:]), z.dtype)
                 for z in zero_outs]
        outs = sharded(*concat_in, *zeros)
        arr = np.asarray(outs[0]).reshape(n_cores, *out_avals[0].shape)
        return [{out_names[0]: arr[c]} for c in range(n_cores)]

    def preload(in_maps):
        import jax
        staged["dev"] = [jax.device_put(a) for a in _concat(in_maps)]
        for a in staged["dev"]:
            a.block_until_ready()

    run.preload = preload
    return run


def kernel(**inputs):
    terms = _cache.get("terms", 1)
    if "run" not in _cache:
        nc = _build(terms)
        _cache["nc"] = nc
        _cache["run"] = _make_runner(nc)
    in_maps = _host_prep(**inputs, terms=terms)
    results = _cache["run"](in_maps)
    out = np.zeros((B, OUT), np.float32)
    for c in range(8):
        hb = c % 2
        out[hb * BH:(hb + 1) * BH] += results[c]["out"]
    return out
